# revision 3
# baseline (speedup 1.0000x reference)
"""MoE SwiGLU experts (MiniQwen3NextExperts) on 8 TRN2 NeuronCores.

Strategy (expert-parallel, per the sharding hint):
  - Host: route (token, k) pairs by expert, one expert per core. Pad each
    expert's token batch to a common capacity C. Pre-transpose/pack weights
    and activations so every device-side matmul is a plain
    [K=128] x [M=128] x [N<=512] fp32r matmul with operands laid out
    contiguously in SBUF.
  - Device (per core, raw Bass, explicit semaphores - every instruction
    carries at most one semaphore wait, which this walrus requires):
      stage 1: gu^T[o, t] = sum_h Wgu[o, h] * x[t, h]   (o-pairs of gate/up)
               act^T[i, t] = silu(gate) * up            (ACT silu + DVE mul)
      stage 2: y^T[h, t] = sum_i Wdn[h, i] * act^T[i, t]
  - Host: scatter-add per-pair outputs weighted by top_k_weights.

All matmuls use float32r (TF32-like: full fp32 storage, ~1e-4 rel err,
1 cycle/row on the PE at N>=256 - same throughput as bf16).

build_nc(C, tiles, repeat=r) unrolls the full pipeline r times with offset
semaphore counters; test.py uses the r>1 variants to measure HW time via
the slope (removes RPC/dispatch overhead).
"""

import numpy as np

import concourse.bass as bass
import concourse.mybir as mybir
from concourse.bass_utils import run_bass_kernel_spmd

F32 = mybir.dt.float32
F32R = mybir.dt.float32r
BF16 = mybir.dt.bfloat16

# Matmul-operand dtype: bf16 halves input DMA (the kernel is otherwise
# DMA-bound in steady state) at ~3e-3 rel err; f32r (TF32-like, ~2.5e-4)
# available as a fallback via BASS_MOE_F32R=1.
import os
USE_F32R = bool(int(os.environ.get("BASS_MOE_F32R", "0")))
IN_DT = F32R if USE_F32R else BF16

try:
    import ml_dtypes
    NP_IN_DT = np.float32 if USE_F32R else ml_dtypes.bfloat16
except ImportError:
    assert USE_F32R, "ml_dtypes needed for bf16 path"
    NP_IN_DT = np.float32

E = 8          # experts == cores
H = 2048       # hidden
I = 1024       # moe intermediate
TOKS = 4096
TOPK = 2
P = 128
NCH_H = H // P     # 16 contraction chunks over hidden
NBLK_J = I // P    # 8 gate/up block pairs
NCH_I = I // P     # 8 contraction chunks over intermediate
NBLK_HT = H // P   # 16 output blocks over hidden


def _t_tiles(C):
    """Split C into near-equal free-dim tiles <=512 (PSUM bank width).

    bf16 matmuls run 1 cycle/row at any N, so tile sizes only need to fit
    PSUM; near-equal tiles keep every tile >=256 for the f32r fallback."""
    assert C % 16 == 0 and C >= 256
    n = -(-C // 512)
    base = -(-(C // n) // 16) * 16
    sizes = [base] * (n - 1) + [C - base * (n - 1)]
    tiles = []
    t0 = 0
    for tn in sizes:
        assert 0 < tn <= 512
        tiles.append((t0, tn))
        t0 += tn
    return tiles


NWGU = 3
NWDN = 4


STOPWATCH_CYC = 1024       # gpsimd tick scratch-memset width


def build_nc(C, tiles, repeat=1, stopwatch=0):
    """stopwatch > 0: run a gpsimd tick counter (that many ticks) and snapshot
    it into probe_out at each iteration end - on-device timing, host-calibrated."""
    T = len(tiles)
    S1 = NBLK_J * T          # stage-1 (j, tt) groups; 2 pe incs each
    S2 = NBLK_HT * T         # stage-2 (ht, tt) groups; 1 pe inc each
    PE_TOT = 2 * S1 + S2     # pe_sem incs per iteration

    nc = bass.Bass("TRN2", target_bir_lowering=False, debug=False, num_devices=E)

    xT = nc.dram_tensor("xT", [NCH_H, P, C], IN_DT, kind="ExternalInput").ap()
    wgu = nc.dram_tensor("wgu", [2 * NBLK_J, P, H], IN_DT, kind="ExternalInput").ap()
    wdn = nc.dram_tensor("wdn", [NBLK_HT, P, I], IN_DT, kind="ExternalInput").ap()
    yT = nc.dram_tensor("yT", [NBLK_HT, P, C], F32, kind="ExternalOutput").ap()
    if stopwatch:
        probe = nc.dram_tensor("probe", [repeat, P, 1], F32,
                               kind="ExternalOutput").ap()
        counter = nc.alloc_sbuf_tensor("counter", [P, 1], F32).ap()
        snap = nc.alloc_sbuf_tensor("snap", [P, 1], F32).ap()
        sw_scratch = nc.alloc_sbuf_tensor("sw_scratch", [P, STOPWATCH_CYC], F32).ap()
        sw_scratch2 = nc.alloc_sbuf_tensor("sw_scratch2", [P, STOPWATCH_CYC], F32).ap()

    x_sb = nc.alloc_sbuf_tensor("x_sb", [P, NCH_H, C], IN_DT).ap()
    act_sb = nc.alloc_sbuf_tensor("act_sb", [P, NCH_I, C], IN_DT).ap()
    wgu_sb = [nc.alloc_sbuf_tensor(f"wgu_sb{b}", [P, 2, NCH_H, P], IN_DT).ap()
              for b in range(NWGU)]
    # all 16 down-proj tiles stay resident (bf16: 2 KB/partition each)
    wdn_sb = [nc.alloc_sbuf_tensor(f"wdn_sb{b}", [P, NCH_I, P], IN_DT).ap()
              for b in range(NBLK_HT)]
    tmp = [nc.alloc_sbuf_tensor(f"tmp{b}", [P, 512], F32).ap() for b in range(2)]
    NOUT = 3
    out_sb = [nc.alloc_sbuf_tensor(f"out_sb{b}", [P, 512], F32).ap()
              for b in range(NOUT)]

    ps_g = [nc.alloc_psum_tensor(f"ps_g{b}", [P, 512], F32).ap() for b in range(2)]
    ps_u = [nc.alloc_psum_tensor(f"ps_u{b}", [P, 512], F32).ap() for b in range(2)]
    NPSY = 3
    ps_y = [nc.alloc_psum_tensor(f"ps_y{b}", [P, 512], F32).ap() for b in range(NPSY)]

    import contextlib
    with contextlib.ExitStack() as ctx:
        block = ctx.enter_context(nc.Block())
        # DMA-completion sems are scoped to a dependency group so that
        # out-of-order completion across HWDGE queues cannot satisfy a wait
        # early: every wait on these sems is for ALL increments of the group.
        dma_xt = [ctx.enter_context(nc.semaphore(f"dma_xt{t}")) for t in range(T)]
        dma_gu = [ctx.enter_context(nc.semaphore(f"dma_gu{j}"))
                  for j in range(NBLK_J)]
        # fill staircase: pair-0 and x-tile-0 land in halves, each with its
        # own sem so the waits stay safe under out-of-order DMA completion
        dma_g0 = [ctx.enter_context(nc.semaphore(f"dma_g0{h}")) for h in range(2)]
        dma_u0 = [ctx.enter_context(nc.semaphore(f"dma_u0{h}")) for h in range(2)]
        dma_x0 = [ctx.enter_context(nc.semaphore(f"dma_x0{h}")) for h in range(2)]
        dma_wd = ctx.enter_context(nc.semaphore("dma_wd"))
        dma_ob = [ctx.enter_context(nc.semaphore(f"dma_ob{b}"))
                  for b in range(NOUT)]
        pe_sem = ctx.enter_context(nc.semaphore("pe_sem"))
        dma_probe = ctx.enter_context(nc.semaphore("dma_probe")) if stopwatch else None
        act1 = ctx.enter_context(nc.semaphore("act1"))
        act2 = ctx.enter_context(nc.semaphore("act2"))
        dve = ctx.enter_context(nc.semaphore("dve"))

        @block.sync
        def _(sync):
            # Loads only - stage-2 stores live on the scalar engine, so
            # iteration it+1's loads overlap iteration it's stage 2.
            for it in range(repeat):
                bp = it * PE_TOT
                if it > 0:
                    # x_sb / wgu_sb free once prior iteration's stage 1 done
                    sync.wait_ge(pe_sem, (it - 1) * PE_TOT + 2 * S1)

                def wgu_pair(j):
                    if j >= NWGU:
                        # wgu_sb[j%NWGU] read by PE until groups of j-NWGU done
                        sync.wait_ge(pe_sem, bp + 2 * T * (j - NWGU + 1))
                    sync.dma_start(wgu_sb[j % NWGU][:, 0],
                                   wgu[j]).then_inc(dma_gu[j], 16)
                    sync.dma_start(wgu_sb[j % NWGU][:, 1],
                                   wgu[NBLK_J + j]).then_inc(dma_gu[j], 16)

                # x arrives tile-by-tile; weight pairs are interleaved so
                # pair j lands before PE finishes pair j-1.
                # Fill staircase: gate0/x0/up0 in half-granular pieces so
                # PE starts after ~1.5 MB. Only pairs j < NWGU may interleave
                # with the x stream: a guarded pair (j >= NWGU) issued before
                # x tiles that earlier stage-1 groups still need would
                # deadlock the SP stream.
                t0_0, tn_0 = tiles[0]
                for h in range(2):
                    ch0 = 8 * h
                    sync.dma_start(wgu_sb[0][:, 0, ch0:ch0 + 8, :],
                                   wgu[0][:, 1024 * h:1024 * h + 1024]
                                   ).then_inc(dma_g0[h], 16)
                    for c in range(ch0, ch0 + 8):
                        sync.dma_start(x_sb[:, c, t0_0:t0_0 + tn_0],
                                       xT[c][:, t0_0:t0_0 + tn_0]
                                       ).then_inc(dma_x0[h], 16)
                for h in range(2):
                    sync.dma_start(wgu_sb[0][:, 1, 8 * h:8 * h + 8, :],
                                   wgu[NBLK_J][:, 1024 * h:1024 * h + 1024]
                                   ).then_inc(dma_u0[h], 16)
                for tt, (t0, tn) in enumerate(tiles):
                    if tt > 0:
                        for c in range(NCH_H):
                            sync.dma_start(x_sb[:, c, t0:t0 + tn],
                                           xT[c][:, t0:t0 + tn]
                                           ).then_inc(dma_xt[tt], 16)
                for j in range(1, NBLK_J):
                    wgu_pair(j)
                if it > 0:
                    # wdn_sb read by prior stage 2 until it fully drains
                    sync.wait_ge(pe_sem, bp)
                for h in range(NBLK_HT):
                    sync.dma_start(wdn_sb[h][:], wdn[h]).then_inc(dma_wd, 16)

        @block.tensor
        def _(tensor):
            for it in range(repeat):
                bp = it * PE_TOT
                b1 = it * S1
                b2 = it * S2
                g = 0
                for j in range(NBLK_J):
                    if j > 0:
                        tensor.wait_ge(dma_gu[j], 32 * (it + 1))
                    for tt, (t0, tn) in enumerate(tiles):
                        first = (j == 0 and tt == 0)
                        if j == 0 and tt > 0:
                            tensor.wait_ge(dma_xt[tt], 16 * NCH_H * (it + 1))
                        if it > 0 or g >= 2:
                            tensor.wait_ge(dve, b1 + g - 1)  # ps_g/u[g%2] free
                        for c in range(NCH_H):
                            if first and c % 8 == 0:
                                h = c // 8
                                tensor.wait_ge(dma_g0[h], 16 * (it + 1))
                                tensor.wait_ge(dma_x0[h], 16 * 8 * (it + 1))
                            mm = tensor.matmul(
                                ps_g[g % 2][:, :tn],
                                wgu_sb[j % NWGU][:, 0, c, :],
                                x_sb[:, c, t0:t0 + tn],
                                start=(c == 0), stop=(c == NCH_H - 1),
                            )
                        mm.then_inc(pe_sem, 1)
                        for c in range(NCH_H):
                            if first and c % 8 == 0:
                                tensor.wait_ge(dma_u0[c // 8], 16 * (it + 1))
                            mm = tensor.matmul(
                                ps_u[g % 2][:, :tn],
                                wgu_sb[j % NWGU][:, 1, c, :],
                                x_sb[:, c, t0:t0 + tn],
                                start=(c == 0), stop=(c == NCH_H - 1),
                            )
                        mm.then_inc(pe_sem, 1)
                        g += 1
                tensor.wait_ge(dve, b1 + S1)              # all act ready
                tensor.wait_ge(dma_wd, 16 * NBLK_HT * (it + 1))  # all wdn tiles
                g2 = 0
                for ht in range(NBLK_HT):
                    for (t0, tn) in tiles:
                        if it > 0 or g2 >= NPSY:
                            tensor.wait_ge(act2, b2 + g2 - NPSY + 1)  # ps_y free
                        for c in range(NCH_I):
                            mm = tensor.matmul(
                                ps_y[g2 % NPSY][:, :tn],
                                wdn_sb[ht][:, c, :],
                                act_sb[:, c, t0:t0 + tn],
                                start=(c == 0), stop=(c == NCH_I - 1),
                            )
                        mm.then_inc(pe_sem, 1)
                        g2 += 1

        @block.scalar
        def _(scalar):
            store_cnt = [0] * NOUT
            for it in range(repeat):
                bp = it * PE_TOT
                b1 = it * S1
                g = 0
                for j in range(NBLK_J):
                    for (t0, tn) in tiles:
                        scalar.wait_ge(pe_sem, bp + 2 * g + 1)
                        if it > 0 or g >= 2:
                            scalar.wait_ge(dve, b1 + g - 1)   # tmp[g%2] free
                        scalar.activation(
                            tmp[g % 2][:, :tn], ps_g[g % 2][:, :tn],
                            mybir.ActivationFunctionType.Silu,
                        ).then_inc(act1, 1)
                        g += 1
                g2 = 0
                for ht in range(NBLK_HT):
                    for (t0, tn) in tiles:
                        b = g2 % NOUT
                        scalar.wait_ge(pe_sem, bp + 2 * S1 + g2 + 1)
                        if store_cnt[b] > 0:
                            scalar.wait_ge(dma_ob[b], 16 * store_cnt[b])
                        scalar.copy(out_sb[b][:, :tn],
                                    ps_y[g2 % NPSY][:, :tn]).then_inc(act2, 1)
                        scalar.dma_start(yT[ht][:, t0:t0 + tn],
                                         out_sb[b][:, :tn]
                                         ).then_inc(dma_ob[b], 16)
                        store_cnt[b] += 1
                        g2 += 1
                if stopwatch:
                    scalar.wait_ge(pe_sem, (it + 1) * PE_TOT)
                    scalar.copy(snap, counter)
                    scalar.dma_start(probe[it], snap).then_inc(dma_probe, 16)
            for b in range(NOUT):
                if store_cnt[b] > 0:
                    scalar.wait_ge(dma_ob[b], 16 * store_cnt[b])
            if stopwatch:
                scalar.wait_ge(dma_probe, 16 * repeat)

        if stopwatch:
            @block.gpsimd
            def _(gpsimd):
                for i in range(stopwatch):
                    gpsimd.tensor_copy(sw_scratch2, sw_scratch)
                    gpsimd.memset(counter, float(i))

        @block.vector
        def _(vector):
            for it in range(repeat):
                bp = it * PE_TOT
                b1 = it * S1
                if it > 0:
                    # act_sb read by prior stage 2 until it fully drains
                    vector.wait_ge(pe_sem, bp)
                g = 0
                for j in range(NBLK_J):
                    for (t0, tn) in tiles:
                        vector.wait_ge(act1, b1 + g + 1)
                        vector.wait_ge(pe_sem, bp + 2 * g + 2)
                        vector.tensor_mul(
                            act_sb[:, j, t0:t0 + tn],
                            tmp[g % 2][:, :tn],
                            ps_u[g % 2][:, :tn],
                        ).then_inc(dve, 1)
                        g += 1

    return nc


_NC_CACHE = {}


def _get_nc(C, tiles, repeat=1):
    key = (C, tuple(tiles), repeat)
    if key not in _NC_CACHE:
        _NC_CACHE[key] = build_nc(C, tiles, repeat)
    return _NC_CACHE[key]


def _route(top_k_index):
    """Return per-expert (token, k) lists and padded capacity."""
    idx = np.asarray(top_k_index)
    tok_t = [[] for _ in range(E)]
    tok_k = [[] for _ in range(E)]
    for k in range(TOPK):
        col = idx[:, k].astype(np.int64)
        for e in range(E):
            ts = np.nonzero(col == e)[0]
            tok_t[e].append(ts)
            tok_k[e].append(np.full(ts.shape, k, np.int64))
    tok_t = [np.concatenate(v) for v in tok_t]
    tok_k = [np.concatenate(v) for v in tok_k]
    counts = np.array([len(v) for v in tok_t])
    cmax = max(int(counts.max()), 256)
    # pad only to 16 (DMA line alignment) - PE cost scales with C, so the
    # old 128-granular padding wasted up to 12% of the matmul work
    C = ((cmax + 15) // 16) * 16
    return tok_t, tok_k, C


def _pack_weights(gate_up_proj, down_proj):
    """Pack per-expert weights into the SBUF tile layouts (contiguous DMAs)."""
    wgu_all = np.empty((E, 2 * NBLK_J, P, H), NP_IN_DT)
    wdn_all = np.empty((E, NBLK_HT, P, I), NP_IN_DT)
    for e in range(E):
        A = np.asarray(gate_up_proj[e], np.float32)          # [2I, H]
        # wgu[j, p, c*128+m] = A[j*128+m, c*128+p]
        wgu_all[e] = (A.reshape(2 * NBLK_J, P, NCH_H, P)
                        .transpose(0, 3, 2, 1)
                        .reshape(2 * NBLK_J, P, H))
        D = np.asarray(down_proj[e], np.float32)             # [H, I]
        # wdn[ht, p, c*128+m] = D[ht*128+m, c*128+p]
        wdn_all[e] = (D.reshape(NBLK_HT, P, NCH_I, P)
                        .transpose(0, 3, 2, 1)
                        .reshape(NBLK_HT, P, I))
    return wgu_all, wdn_all


def kernel(hidden_states, top_k_index, top_k_weights, gate_up_proj, down_proj):
    assert not USE_F32R or True  # f32r fallback shares the bf16 SBUF budget check in build_nc

    hidden_states = np.asarray(hidden_states, np.float32)
    top_k_weights = np.asarray(top_k_weights, np.float32)

    tok_t, tok_k, C = _route(top_k_index)
    tiles = _t_tiles(C)
    nc = _get_nc(C, tiles)

    wgu_all, wdn_all = _pack_weights(gate_up_proj, down_proj)

    in_maps = []
    for e in range(E):
        n_e = len(tok_t[e])
        xe = np.zeros((H, C), NP_IN_DT)
        if n_e:
            xe[:, :n_e] = hidden_states[tok_t[e]].T.astype(NP_IN_DT)
        in_maps.append({
            "xT": np.ascontiguousarray(xe.reshape(NCH_H, P, C)),
            "wgu": wgu_all[e],
            "wdn": wdn_all[e],
        })

    res = run_bass_kernel_spmd(nc, in_maps, core_ids=list(range(E)))

    y_pair = np.zeros((TOKS, TOPK, H), np.float32)
    for e in range(E):
        n_e = len(tok_t[e])
        if n_e == 0:
            continue
        yT = res.results[e]["yT"]                    # [16, 128, C]
        y_e = yT.transpose(2, 0, 1).reshape(C, H)[:n_e]
        y_pair[tok_t[e], tok_k[e]] = y_e
    out = np.einsum("tkh,tk->th", y_pair, top_k_weights).astype(np.float32)
    return out



# revision 4
# speedup vs baseline: 1.0067x; 1.0067x over previous
"""MoE SwiGLU experts (MiniQwen3NextExperts) on 8 TRN2 NeuronCores.

Strategy (expert-parallel, per the sharding hint):
  - Host: route (token, k) pairs by expert, one expert per core. Pad each
    expert's token batch to a common capacity C. Pre-transpose/pack weights
    and activations so every device-side matmul is a plain
    [K=128] x [M=128] x [N<=512] fp32r matmul with operands laid out
    contiguously in SBUF.
  - Device (per core, raw Bass, explicit semaphores - every instruction
    carries at most one semaphore wait, which this walrus requires):
      stage 1: gu^T[o, t] = sum_h Wgu[o, h] * x[t, h]   (o-pairs of gate/up)
               act^T[i, t] = silu(gate) * up            (ACT silu + DVE mul)
      stage 2: y^T[h, t] = sum_i Wdn[h, i] * act^T[i, t]
  - Host: scatter-add per-pair outputs weighted by top_k_weights.

All matmuls use float32r (TF32-like: full fp32 storage, ~1e-4 rel err,
1 cycle/row on the PE at N>=256 - same throughput as bf16).

build_nc(C, tiles, repeat=r) unrolls the full pipeline r times with offset
semaphore counters; test.py uses the r>1 variants to measure HW time via
the slope (removes RPC/dispatch overhead).
"""

import numpy as np

import concourse.bass as bass
import concourse.mybir as mybir
from concourse.bass_utils import run_bass_kernel_spmd

F32 = mybir.dt.float32
F32R = mybir.dt.float32r
BF16 = mybir.dt.bfloat16

F16 = mybir.dt.float16

# Matmul-operand dtype: fp16 halves input DMA vs f32 and runs 1 cycle/row
# on the PE (same rate as bf16) but carries 3 more mantissa bits — all
# values here are well inside fp16 range (|x| < 30, |w| ~ 0.02), so fp16
# strictly dominates bf16 for this problem. f32r (TF32-like, ~2.5e-4)
# remains available via BASS_MOE_F32R=1.
import os
USE_F32R = bool(int(os.environ.get("BASS_MOE_F32R", "0")))
IN_DT = F32R if USE_F32R else F16
NP_IN_DT = np.float32 if USE_F32R else np.float16

E = 8          # experts == cores
H = 2048       # hidden
I = 1024       # moe intermediate
TOKS = 4096
TOPK = 2
P = 128
NCH_H = H // P     # 16 contraction chunks over hidden
NBLK_J = I // P    # 8 gate/up block pairs
NCH_I = I // P     # 8 contraction chunks over intermediate
NBLK_HT = H // P   # 16 output blocks over hidden


def _t_tiles(C):
    """Split C into near-equal free-dim tiles <=512 (PSUM bank width).

    bf16 matmuls run 1 cycle/row at any N, so tile sizes only need to fit
    PSUM; near-equal tiles keep every tile >=256 for the f32r fallback."""
    assert C % 16 == 0 and C >= 256
    n = -(-C // 512)
    base = -(-(C // n) // 16) * 16
    sizes = [base] * (n - 1) + [C - base * (n - 1)]
    tiles = []
    t0 = 0
    for tn in sizes:
        assert 0 < tn <= 512
        tiles.append((t0, tn))
        t0 += tn
    return tiles


NWGU = 3
NWDN = 4


STOPWATCH_CYC = 1024       # gpsimd tick scratch-memset width


def build_nc(C, tiles, repeat=1, stopwatch=0):
    """stopwatch > 0: run a gpsimd tick counter (that many ticks) and snapshot
    it into probe_out at each iteration end - on-device timing, host-calibrated."""
    T = len(tiles)
    S1 = NBLK_J * T          # stage-1 (j, tt) groups; 2 pe incs each
    S2 = NBLK_HT * T         # stage-2 (ht, tt) groups; 1 pe inc each
    PE_TOT = 2 * S1 + S2     # pe_sem incs per iteration

    nc = bass.Bass("TRN2", target_bir_lowering=False, debug=False, num_devices=E)

    xT = nc.dram_tensor("xT", [NCH_H, P, C], IN_DT, kind="ExternalInput").ap()
    wgu = nc.dram_tensor("wgu", [2 * NBLK_J, P, H], IN_DT, kind="ExternalInput").ap()
    wdn = nc.dram_tensor("wdn", [NBLK_HT, P, I], IN_DT, kind="ExternalInput").ap()
    yT = nc.dram_tensor("yT", [NBLK_HT, P, C], F32, kind="ExternalOutput").ap()
    if stopwatch:
        probe = nc.dram_tensor("probe", [repeat, P, 1], F32,
                               kind="ExternalOutput").ap()
        counter = nc.alloc_sbuf_tensor("counter", [P, 1], F32).ap()
        snap = nc.alloc_sbuf_tensor("snap", [P, 1], F32).ap()
        sw_scratch = nc.alloc_sbuf_tensor("sw_scratch", [P, STOPWATCH_CYC], F32).ap()
        sw_scratch2 = nc.alloc_sbuf_tensor("sw_scratch2", [P, STOPWATCH_CYC], F32).ap()

    x_sb = nc.alloc_sbuf_tensor("x_sb", [P, NCH_H, C], IN_DT).ap()
    act_sb = nc.alloc_sbuf_tensor("act_sb", [P, NCH_I, C], IN_DT).ap()
    wgu_sb = [nc.alloc_sbuf_tensor(f"wgu_sb{b}", [P, 2, NCH_H, P], IN_DT).ap()
              for b in range(NWGU)]
    # all 16 down-proj tiles stay resident (bf16: 2 KB/partition each)
    wdn_sb = [nc.alloc_sbuf_tensor(f"wdn_sb{b}", [P, NCH_I, P], IN_DT).ap()
              for b in range(NBLK_HT)]
    tmp = [nc.alloc_sbuf_tensor(f"tmp{b}", [P, 512], F32).ap() for b in range(2)]
    NOUT = 3
    out_sb = [nc.alloc_sbuf_tensor(f"out_sb{b}", [P, 512], F32).ap()
              for b in range(NOUT)]

    ps_g = [nc.alloc_psum_tensor(f"ps_g{b}", [P, 512], F32).ap() for b in range(2)]
    ps_u = [nc.alloc_psum_tensor(f"ps_u{b}", [P, 512], F32).ap() for b in range(2)]
    NPSY = 3
    ps_y = [nc.alloc_psum_tensor(f"ps_y{b}", [P, 512], F32).ap() for b in range(NPSY)]

    import contextlib
    with contextlib.ExitStack() as ctx:
        block = ctx.enter_context(nc.Block())
        # DMA-completion sems are scoped to a dependency group so that
        # out-of-order completion across HWDGE queues cannot satisfy a wait
        # early: every wait on these sems is for ALL increments of the group.
        dma_xt = [ctx.enter_context(nc.semaphore(f"dma_xt{t}")) for t in range(T)]
        dma_gu = [ctx.enter_context(nc.semaphore(f"dma_gu{j}"))
                  for j in range(NBLK_J)]
        # fill staircase: pair-0 and x-tile-0 land in halves, each with its
        # own sem so the waits stay safe under out-of-order DMA completion
        dma_g0 = [ctx.enter_context(nc.semaphore(f"dma_g0{h}")) for h in range(2)]
        dma_u0 = [ctx.enter_context(nc.semaphore(f"dma_u0{h}")) for h in range(2)]
        dma_x0 = [ctx.enter_context(nc.semaphore(f"dma_x0{h}")) for h in range(2)]
        dma_wd = ctx.enter_context(nc.semaphore("dma_wd"))
        dma_ob = [ctx.enter_context(nc.semaphore(f"dma_ob{b}"))
                  for b in range(NOUT)]
        pe_sem = ctx.enter_context(nc.semaphore("pe_sem"))
        dma_probe = ctx.enter_context(nc.semaphore("dma_probe")) if stopwatch else None
        act1 = ctx.enter_context(nc.semaphore("act1"))
        act2 = ctx.enter_context(nc.semaphore("act2"))
        dve = ctx.enter_context(nc.semaphore("dve"))

        @block.sync
        def _(sync):
            # Loads only - stage-2 stores live on the scalar engine, so
            # iteration it+1's loads overlap iteration it's stage 2.
            for it in range(repeat):
                bp = it * PE_TOT
                if it > 0:
                    # x_sb / wgu_sb free once prior iteration's stage 1 done
                    sync.wait_ge(pe_sem, (it - 1) * PE_TOT + 2 * S1)

                def wgu_pair(j):
                    if j >= NWGU:
                        # wgu_sb[j%NWGU] read by PE until groups of j-NWGU done
                        sync.wait_ge(pe_sem, bp + 2 * T * (j - NWGU + 1))
                    sync.dma_start(wgu_sb[j % NWGU][:, 0],
                                   wgu[j]).then_inc(dma_gu[j], 16)
                    sync.dma_start(wgu_sb[j % NWGU][:, 1],
                                   wgu[NBLK_J + j]).then_inc(dma_gu[j], 16)

                # x arrives tile-by-tile; weight pairs are interleaved so
                # pair j lands before PE finishes pair j-1.
                # Fill staircase: gate0/x0/up0 in half-granular pieces so
                # PE starts after ~1.5 MB. Only pairs j < NWGU may interleave
                # with the x stream: a guarded pair (j >= NWGU) issued before
                # x tiles that earlier stage-1 groups still need would
                # deadlock the SP stream.
                t0_0, tn_0 = tiles[0]
                for h in range(2):
                    ch0 = 8 * h
                    sync.dma_start(wgu_sb[0][:, 0, ch0:ch0 + 8, :],
                                   wgu[0][:, 1024 * h:1024 * h + 1024]
                                   ).then_inc(dma_g0[h], 16)
                    for c in range(ch0, ch0 + 8):
                        sync.dma_start(x_sb[:, c, t0_0:t0_0 + tn_0],
                                       xT[c][:, t0_0:t0_0 + tn_0]
                                       ).then_inc(dma_x0[h], 16)
                for h in range(2):
                    sync.dma_start(wgu_sb[0][:, 1, 8 * h:8 * h + 8, :],
                                   wgu[NBLK_J][:, 1024 * h:1024 * h + 1024]
                                   ).then_inc(dma_u0[h], 16)
                for tt, (t0, tn) in enumerate(tiles):
                    if tt > 0:
                        for c in range(NCH_H):
                            sync.dma_start(x_sb[:, c, t0:t0 + tn],
                                           xT[c][:, t0:t0 + tn]
                                           ).then_inc(dma_xt[tt], 16)
                for j in range(1, NBLK_J):
                    wgu_pair(j)
                if it > 0:
                    # wdn_sb read by prior stage 2 until it fully drains
                    sync.wait_ge(pe_sem, bp)
                for h in range(NBLK_HT):
                    sync.dma_start(wdn_sb[h][:], wdn[h]).then_inc(dma_wd, 16)

        @block.tensor
        def _(tensor):
            for it in range(repeat):
                bp = it * PE_TOT
                b1 = it * S1
                b2 = it * S2
                g = 0
                for j in range(NBLK_J):
                    if j > 0:
                        tensor.wait_ge(dma_gu[j], 32 * (it + 1))
                    for tt, (t0, tn) in enumerate(tiles):
                        first = (j == 0 and tt == 0)
                        if j == 0 and tt > 0:
                            tensor.wait_ge(dma_xt[tt], 16 * NCH_H * (it + 1))
                        if it > 0 or g >= 2:
                            tensor.wait_ge(dve, b1 + g - 1)  # ps_g/u[g%2] free
                        for c in range(NCH_H):
                            if first and c % 8 == 0:
                                h = c // 8
                                tensor.wait_ge(dma_g0[h], 16 * (it + 1))
                                tensor.wait_ge(dma_x0[h], 16 * 8 * (it + 1))
                            mm = tensor.matmul(
                                ps_g[g % 2][:, :tn],
                                wgu_sb[j % NWGU][:, 0, c, :],
                                x_sb[:, c, t0:t0 + tn],
                                start=(c == 0), stop=(c == NCH_H - 1),
                            )
                        mm.then_inc(pe_sem, 1)
                        for c in range(NCH_H):
                            if first and c % 8 == 0:
                                tensor.wait_ge(dma_u0[c // 8], 16 * (it + 1))
                            mm = tensor.matmul(
                                ps_u[g % 2][:, :tn],
                                wgu_sb[j % NWGU][:, 1, c, :],
                                x_sb[:, c, t0:t0 + tn],
                                start=(c == 0), stop=(c == NCH_H - 1),
                            )
                        mm.then_inc(pe_sem, 1)
                        g += 1
                tensor.wait_ge(dve, b1 + S1)              # all act ready
                tensor.wait_ge(dma_wd, 16 * NBLK_HT * (it + 1))  # all wdn tiles
                g2 = 0
                for ht in range(NBLK_HT):
                    for (t0, tn) in tiles:
                        if it > 0 or g2 >= NPSY:
                            tensor.wait_ge(act2, b2 + g2 - NPSY + 1)  # ps_y free
                        for c in range(NCH_I):
                            mm = tensor.matmul(
                                ps_y[g2 % NPSY][:, :tn],
                                wdn_sb[ht][:, c, :],
                                act_sb[:, c, t0:t0 + tn],
                                start=(c == 0), stop=(c == NCH_I - 1),
                            )
                        mm.then_inc(pe_sem, 1)
                        g2 += 1

        @block.scalar
        def _(scalar):
            store_cnt = [0] * NOUT
            for it in range(repeat):
                bp = it * PE_TOT
                b1 = it * S1
                g = 0
                for j in range(NBLK_J):
                    for (t0, tn) in tiles:
                        scalar.wait_ge(pe_sem, bp + 2 * g + 1)
                        if it > 0 or g >= 2:
                            scalar.wait_ge(dve, b1 + g - 1)   # tmp[g%2] free
                        scalar.activation(
                            tmp[g % 2][:, :tn], ps_g[g % 2][:, :tn],
                            mybir.ActivationFunctionType.Silu,
                        ).then_inc(act1, 1)
                        g += 1
                g2 = 0
                for ht in range(NBLK_HT):
                    for (t0, tn) in tiles:
                        b = g2 % NOUT
                        scalar.wait_ge(pe_sem, bp + 2 * S1 + g2 + 1)
                        if store_cnt[b] > 0:
                            scalar.wait_ge(dma_ob[b], 16 * store_cnt[b])
                        scalar.copy(out_sb[b][:, :tn],
                                    ps_y[g2 % NPSY][:, :tn]).then_inc(act2, 1)
                        scalar.dma_start(yT[ht][:, t0:t0 + tn],
                                         out_sb[b][:, :tn]
                                         ).then_inc(dma_ob[b], 16)
                        store_cnt[b] += 1
                        g2 += 1
                if stopwatch:
                    scalar.wait_ge(pe_sem, (it + 1) * PE_TOT)
                    scalar.copy(snap, counter)
                    scalar.dma_start(probe[it], snap).then_inc(dma_probe, 16)
            for b in range(NOUT):
                if store_cnt[b] > 0:
                    scalar.wait_ge(dma_ob[b], 16 * store_cnt[b])
            if stopwatch:
                scalar.wait_ge(dma_probe, 16 * repeat)

        if stopwatch:
            @block.gpsimd
            def _(gpsimd):
                for i in range(stopwatch):
                    gpsimd.tensor_copy(sw_scratch2, sw_scratch)
                    gpsimd.memset(counter, float(i))

        @block.vector
        def _(vector):
            for it in range(repeat):
                bp = it * PE_TOT
                b1 = it * S1
                if it > 0:
                    # act_sb read by prior stage 2 until it fully drains
                    vector.wait_ge(pe_sem, bp)
                g = 0
                for j in range(NBLK_J):
                    for (t0, tn) in tiles:
                        vector.wait_ge(act1, b1 + g + 1)
                        vector.wait_ge(pe_sem, bp + 2 * g + 2)
                        vector.tensor_mul(
                            act_sb[:, j, t0:t0 + tn],
                            tmp[g % 2][:, :tn],
                            ps_u[g % 2][:, :tn],
                        ).then_inc(dve, 1)
                        g += 1

    return nc


_NC_CACHE = {}


def _get_nc(C, tiles, repeat=1):
    key = (C, tuple(tiles), repeat)
    if key not in _NC_CACHE:
        _NC_CACHE[key] = build_nc(C, tiles, repeat)
    return _NC_CACHE[key]


def _route(top_k_index):
    """Return per-expert (token, k) lists and padded capacity."""
    idx = np.asarray(top_k_index)
    tok_t = [[] for _ in range(E)]
    tok_k = [[] for _ in range(E)]
    for k in range(TOPK):
        col = idx[:, k].astype(np.int64)
        for e in range(E):
            ts = np.nonzero(col == e)[0]
            tok_t[e].append(ts)
            tok_k[e].append(np.full(ts.shape, k, np.int64))
    tok_t = [np.concatenate(v) for v in tok_t]
    tok_k = [np.concatenate(v) for v in tok_k]
    counts = np.array([len(v) for v in tok_t])
    cmax = max(int(counts.max()), 256)
    # pad only to 16 (DMA line alignment) - PE cost scales with C, so the
    # old 128-granular padding wasted up to 12% of the matmul work
    C = ((cmax + 15) // 16) * 16
    return tok_t, tok_k, C


def _pack_weights(gate_up_proj, down_proj):
    """Pack per-expert weights into the SBUF tile layouts (contiguous DMAs)."""
    wgu_all = np.empty((E, 2 * NBLK_J, P, H), NP_IN_DT)
    wdn_all = np.empty((E, NBLK_HT, P, I), NP_IN_DT)
    for e in range(E):
        A = np.asarray(gate_up_proj[e], np.float32)          # [2I, H]
        # wgu[j, p, c*128+m] = A[j*128+m, c*128+p]
        wgu_all[e] = (A.reshape(2 * NBLK_J, P, NCH_H, P)
                        .transpose(0, 3, 2, 1)
                        .reshape(2 * NBLK_J, P, H))
        D = np.asarray(down_proj[e], np.float32)             # [H, I]
        # wdn[ht, p, c*128+m] = D[ht*128+m, c*128+p]
        wdn_all[e] = (D.reshape(NBLK_HT, P, NCH_I, P)
                        .transpose(0, 3, 2, 1)
                        .reshape(NBLK_HT, P, I))
    return wgu_all, wdn_all


def kernel(hidden_states, top_k_index, top_k_weights, gate_up_proj, down_proj):
    assert not USE_F32R or True  # f32r fallback shares the bf16 SBUF budget check in build_nc

    hidden_states = np.asarray(hidden_states, np.float32)
    top_k_weights = np.asarray(top_k_weights, np.float32)

    tok_t, tok_k, C = _route(top_k_index)
    tiles = _t_tiles(C)
    nc = _get_nc(C, tiles)

    wgu_all, wdn_all = _pack_weights(gate_up_proj, down_proj)

    in_maps = []
    for e in range(E):
        n_e = len(tok_t[e])
        xe = np.zeros((H, C), NP_IN_DT)
        if n_e:
            xe[:, :n_e] = hidden_states[tok_t[e]].T.astype(NP_IN_DT)
        in_maps.append({
            "xT": np.ascontiguousarray(xe.reshape(NCH_H, P, C)),
            "wgu": wgu_all[e],
            "wdn": wdn_all[e],
        })

    res = run_bass_kernel_spmd(nc, in_maps, core_ids=list(range(E)))

    y_pair = np.zeros((TOKS, TOPK, H), np.float32)
    for e in range(E):
        n_e = len(tok_t[e])
        if n_e == 0:
            continue
        yT = res.results[e]["yT"]                    # [16, 128, C]
        y_e = yT.transpose(2, 0, 1).reshape(C, H)[:n_e]
        y_pair[tok_t[e], tok_k[e]] = y_e
    out = np.einsum("tkh,tk->th", y_pair, top_k_weights).astype(np.float32)
    return out

